# revision 18
# baseline (speedup 1.0000x reference)
"""Trainium2 Bass kernel for nn_MultiHeadAttention_90993177133622.

Math (from reference):
  Q = QKV @ Wq.T + bq   (same for K, V)            [B,S,E] -> view [B,S,H,D]
  P[b,s,h,g] = sum_d Q[b,s,h,d] K[b,s,g,d] * sqrt(D)   (per-token [H,H] attn)
  causal tril mask over [H,H], softmax over g
  out = (P @ V).reshape(B,S,E) @ Wo.T + bo

Sharding: data-parallel over the 16384 tokens across 8 cores (2048 each).
Device layout: token-major [128 tokens, E] tiles; projections run on the PE
(activation-stationary, float32r), per-token attention on DVE/ACT.
"""
import sys, os, types, ctypes, contextlib, json, math

sys.path.insert(0, "/opt/trn_rl_repo")
import numpy as np

B, S, E, H, D = 4, 4096, 1024, 16, 64
NCORES = 8
TOK = B * S                 # 16384
TPC = TOK // NCORES         # tokens per core: 2048
CHUNK = 128                 # tokens per tile
NCH = TPC // CHUNK          # 16 chunks per core
KC = E // 128               # 8 contraction chunks
SCALE = math.sqrt(D)        # reference MULTIPLIES by sqrt(D)
NEG = -1.0e30


# ---------------------------------------------------------------- infra shims
def _install_ntff_hook():
    """antenv.axon_hooks is missing in this image; provide it so
    run_bass_kernel_spmd(trace=True) can profile via libaxon_pjrt."""
    if "antenv.axon_hooks" in sys.modules:
        return
    mod = types.ModuleType("antenv.axon_hooks")
    state = {"hook": None}
    mod.set_axon_ntff_profile_hook = lambda h: state.__setitem__("hook", h)
    mod.get_axon_ntff_profile_hook = lambda: state["hook"]
    sys.modules["antenv.axon_hooks"] = mod
    try:
        lib = ctypes.CDLL("/opt/axon/libaxon_pjrt.so")
    except OSError:
        return
    if not hasattr(lib, "axon_start_nrt_profile"):
        return
    lib.axon_start_nrt_profile.argtypes = [ctypes.POINTER(ctypes.c_int64), ctypes.c_size_t]
    lib.axon_start_nrt_profile.restype = ctypes.c_int64
    lib.axon_stop_nrt_profile.argtypes = [ctypes.c_char_p]
    lib.axon_stop_nrt_profile.restype = ctypes.c_int64

    @contextlib.contextmanager
    def _hook(output_dir, device_ids):
        import jax
        jax.devices()
        if device_ids:
            ids = (ctypes.c_int64 * len(device_ids))(*device_ids)
            rc = lib.axon_start_nrt_profile(ids, len(device_ids))
        else:
            rc = lib.axon_start_nrt_profile(None, 0)
        if rc != 0:
            raise RuntimeError(f"axon_start_nrt_profile rc={rc}")
        try:
            yield
        finally:
            n = lib.axon_stop_nrt_profile(str(output_dir).encode())
            print(f"profile: {n} file(s) -> {output_dir}", file=sys.stderr)

    mod.set_axon_ntff_profile_hook(_hook)


_install_ntff_hook()

_MAX_WAITS = 2


def _split_waits_json(raw: bytes) -> bytes:
    """This walrus build rejects CTRL instructions with >2 sync waits; split
    extra waits off Drain/EventSemaphore into preceding wait-only ctrls."""
    j = json.loads(raw)
    for fn in j["functions"]:
        for bb in fn["blocks"]:
            out = []
            for inst in bb["instructions"]:
                si = inst.get("sync_info")
                waits = (si or {}).get("on_wait") or []
                limit = _MAX_WAITS if inst.get("opcode") == "EventSemaphore" else 1
                if len(waits) > limit:
                    head, tail = waits[:-limit], waits[-limit:]
                    for k in range(0, len(head), _MAX_WAITS):
                        out.append({
                            "debug": inst.get("debug", 0),
                            "engine": inst["engine"],
                            "ins": [], "outs": [],
                            "name": inst["name"] + f"_w{k}",
                            "opcode": "EventSemaphore",
                            "sync_info": {"on_update": [], "on_wait": head[k:k + _MAX_WAITS]},
                        })
                    si["on_wait"] = tail
                out.append(inst)
            bb["instructions"] = out
    return json.dumps(j).encode()


def _patch_bass(nc):
    orig = nc.to_json_bytes
    nc.to_json_bytes = lambda: _split_waits_json(orig())
    return nc


# ---------------------------------------------------------------- the program
_cache = {}


def _build(is_causal: bool, use_bias: bool = True):
    import concourse.bass as bass
    import concourse.tile as tile
    import concourse.mybir as mybir
    from contextlib import ExitStack

    f32 = mybir.dt.float32
    f16 = mybir.dt.float16
    Alu = mybir.AluOpType
    Act = mybir.ActivationFunctionType

    nc = bass.Bass("TRN2", target_bir_lowering=False, debug=False, enable_asserts=False)

    xt = nc.dram_tensor("xt", [E, TPC], f16, kind="ExternalInput").ap()
    ws = {n: nc.dram_tensor(n, [E, E], f16, kind="ExternalInput").ap()
          for n in ("wq", "wk", "wv", "wo")}
    bias4 = nc.dram_tensor("bias4", [1, 4 * E], f16, kind="ExternalInput").ap()
    ident = nc.dram_tensor("ident", [128, 128], f16, kind="ExternalInput").ap()
    out_d = nc.dram_tensor("out", [TPC, E], f32, kind="ExternalOutput").ap()

    with tile.TileContext(nc) as tc, ExitStack() as ctx:
        wpool = ctx.enter_context(tc.tile_pool(name="w", bufs=1))
        xpool = ctx.enter_context(tc.tile_pool(name="x", bufs=2))
        qkv = ctx.enter_context(tc.tile_pool(name="qkv", bufs=2))
        ppool = ctx.enter_context(tc.tile_pool(name="p", bufs=1))
        prodp = ctx.enter_context(tc.tile_pool(name="prod", bufs=2))
        treep = ctx.enter_context(tc.tile_pool(name="tree", bufs=2))
        treei = ctx.enter_context(tc.tile_pool(name="treei", bufs=1))
        _ttag = lambda n: f"ts{n}"
        stats = ctx.enter_context(tc.tile_pool(name="st", bufs=2))
        opool = ctx.enter_context(tc.tile_pool(name="o", bufs=1))
        o2pool = ctx.enter_context(tc.tile_pool(name="o2", bufs=2))
        psum = ctx.enter_context(tc.tile_pool(name="ps", bufs=2, space="PSUM"))
        psum1 = ctx.enter_context(tc.tile_pool(name="ps1", bufs=1, space="PSUM"))

        # resident constants
        w_sb = {}
        for n in ("wq", "wk", "wv", "wo"):
            t = wpool.tile([128, KC, E], f16, tag=f"w_{n}")
            nc.sync.dma_start(t[:], ws[n].rearrange("(k p) n -> p k n", p=128))
            w_sb[n] = t
        if use_bias:
            b_sb = wpool.tile([1, 4 * E], f16, tag="bias")
            nc.sync.dma_start(b_sb[:], bias4[:])
            ones = wpool.tile([1, CHUNK], f16, tag="ones")
            nc.vector.memset(ones[:], 1.0)
        id_sb = wpool.tile([128, 128], f16, tag="ident")
        nc.sync.dma_start(id_sb[:], ident[:])

        xt_r = xt.rearrange("(k p) t -> p k t", p=128)

        for ci in range(NCH):
            tsl = slice(ci * CHUNK, (ci + 1) * CHUNK)
            # ---- load X chunk (feature-major lhsT for all projections)
            x_sb = xpool.tile([128, KC, CHUNK], f16, tag="x")
            nc.sync.dma_start(x_sb[:], xt_r[:, :, tsl])

            # ---- Q,K,V projections -> token-major [128 tok, E] fp16
            sb = {}
            for pi, n in enumerate(("wq", "wk", "wv")):
                ps = psum.tile([128, E], f32, tag="proj")
                for k in range(KC):
                    for nn in range(2):
                        nsl = slice(nn * 512, (nn + 1) * 512)
                        nc.tensor.matmul(
                            ps[:, nsl],
                            x_sb[:, k, :],
                            w_sb[n][:, k, nsl],
                            start=(k == 0),
                            stop=(k == KC - 1 and not use_bias))
                if use_bias:
                    for nn in range(2):
                        nsl = slice(nn * 512, (nn + 1) * 512)
                        nc.tensor.matmul(
                            ps[:, nsl],
                            ones[:],
                            b_sb[0:1, pi * E + nn * 512: pi * E + (nn + 1) * 512],
                            start=False, stop=True)
                t = qkv.tile([128, E], f16, tag=n)
                nc.scalar.copy(t[:], ps[:])
                sb[n] = t
            q_sb, k_sb, v_sb = sb["wq"], sb["wk"], sb["wv"]

            # ---- attention (token-major, per-head causal-packed, fp16 TT ops)
            gmax = (lambda h: h + 1) if is_causal else (lambda h: H)
            NP = sum(gmax(h) for h in range(H))      # packed pair count
            off = [0] * (H + 1)
            for h in range(H):
                off[h + 1] = off[h] + gmax(h)

            def tt(eng, out, in0, in1, op):
                return eng.add_instruction(mybir.InstTensorTensor(
                    name=nc.get_next_instruction_name(), op=op,
                    ins=[eng.lower_ap(in0), eng.lower_ap(in1)],
                    outs=[eng.lower_ap(out)]))

            p_t = ppool.tile([128, H * H], f32, tag="praw")
            nc.gpsimd.memset(p_t[:], NEG)
            k_v = k_sb[:].rearrange("p (g d) -> p g d", g=H)
            qp = treep.tile([128, NP * D], f16, tag="big")
            for h in range(H):
                g = gmax(h)
                tt(nc.vector,
                   qp[:, off[h] * D:off[h + 1] * D].rearrange("p (g d) -> p g d", g=g),
                   q_sb[:, h * D:(h + 1) * D].unsqueeze(1).broadcast_to([128, g, D]),
                   k_v[:, :g, :], Alu.mult)
            # bulk pairwise tree over d: 64 -> 2
            lv = qp[:].rearrange("p (n d) -> p n d", n=NP)
            w = D
            for li in range(5):
                w //= 2
                pool_ = treep if li == 0 else treei
                nt = pool_.tile([128, NP * w], f16, tag=_ttag(NP * w))
                tt(nc.gpsimd if li == 0 else nc.vector,
                   nt[:].rearrange("p (n d) -> p n d", n=NP),
                   lv[:, :, 0:w], lv[:, :, w:2 * w], Alu.add)
                lv = nt[:].rearrange("p (n d) -> p n d", n=NP)
            for h in range(H):
                g = gmax(h)
                tt(nc.vector,
                   p_t[:, h * H:h * H + g].unsqueeze(2),
                   lv[:, off[h]:off[h + 1], 0:1], lv[:, off[h]:off[h + 1], 1:2],
                   Alu.add)

            p3 = p_t[:].rearrange("p (h g) -> p h g", h=H)
            mx = stats.tile([128, H], f32, tag="mx")
            nc.vector.tensor_reduce(mx[:], p3, mybir.AxisListType.X, Alu.max)
            p2_t = ppool.tile([128, H * H], f32, tag="psub")
            tt(nc.vector,
               p2_t[:].rearrange("p (h g) -> p h g", h=H),
               p3, mx[:].unsqueeze(2).broadcast_to([128, H, H]), Alu.subtract)
            ex_t = ppool.tile([128, H * H], f16, tag="pexp")
            nc.scalar.activation(ex_t[:], p2_t[:], Act.Exp, scale=float(SCALE))
            sm = stats.tile([128, H], f32, tag="sm")
            nc.vector.tensor_reduce(
                sm[:], ex_t[:].rearrange("p (h g) -> p h g", h=H),
                mybir.AxisListType.X, Alu.add)
            rc = stats.tile([128, H], f32, tag="rc")
            nc.vector.reciprocal(rc[:], sm[:])
            pn_t = ppool.tile([128, H * H], f16, tag="pnorm")
            tt(nc.vector,
               pn_t[:].rearrange("p (h g) -> p h g", h=H),
               ex_t[:].rearrange("p (h g) -> p h g", h=H),
               rc[:].unsqueeze(2).broadcast_to([128, H, H]), Alu.mult)

            attn = opool.tile([128, E], f16, tag="attn")
            v_dg = v_sb[:].rearrange("p (d g) -> p d g", g=H)  # V is d-major
            # padded head groups: head h in group with g-extent Gp (pn is 0 on
            # masked g, so padded products vanish); bulk pairwise tree per group
            if is_causal:
                groups = [(0, 8, 8), (8, 16, 16)]
            else:
                groups = [(0, 16, 16)]
            with nc.allow_low_precision("fp16 attn accumulation over 16 heads"):
                for h0, h1, Gp in groups:
                    nh = h1 - h0
                    gp = treep.tile([128, nh * D * Gp], f16, tag=("big" if Gp == 16 and is_causal else _ttag(nh * D * Gp)))
                    tt(nc.vector,
                       gp[:].rearrange("p (h d g) -> p h d g", h=nh, d=D),
                       pn_t[:, h0 * H:h1 * H]
                       .rearrange("p (h g) -> p h g", h=nh)[:, :, :Gp]
                       .unsqueeze(2).broadcast_to([128, nh, D, Gp]),
                       v_dg[:, :, :Gp].unsqueeze(1)
                       .broadcast_to([128, nh, D, Gp]),
                       Alu.mult)
                    lvv = gp[:].rearrange("p (n g) -> p n g", n=nh * D)
                    w2 = Gp
                    li = 0
                    while w2 > 2:
                        w2 //= 2
                        nt2 = treei.tile([128, nh * D * w2], f16, tag="i" + _ttag(nh * D * w2))
                        tt(nc.vector,
                           nt2[:].rearrange("p (n g) -> p n g", n=nh * D),
                           lvv[:, :, 0:w2], lvv[:, :, w2:2 * w2], Alu.add)
                        lvv = nt2[:].rearrange("p (n g) -> p n g", n=nh * D)
                        li += 1
                    tt(nc.vector,
                       attn[:, h0 * D:h1 * D].unsqueeze(2),
                       lvv[:, :, 0:1], lvv[:, :, 1:2], Alu.add)

            # ---- transpose attn to feature-major for the O projection
            ps_t = psum1.tile([128, E], f16, tag="tr")
            for j in range(KC):
                nc.tensor.transpose(
                    ps_t[:, j * 128:(j + 1) * 128],
                    attn[:, j * 128:(j + 1) * 128], id_sb[:])
            ao = opool.tile([128, KC, CHUNK], f16, tag="ao")
            nc.scalar.copy(ao[:], ps_t[:])

            # ---- O projection -> token-major out chunk
            ps_o = psum1.tile([128, E], f32, tag="oproj")
            for nn in range(2):
                nsl = slice(nn * 512, (nn + 1) * 512)
                for k in range(KC):
                    nc.tensor.matmul(
                        ps_o[:, nsl],
                        ao[:, k, :],
                        w_sb["wo"][:, k, nsl],
                        start=(k == 0),
                        stop=(k == KC - 1 and not use_bias))
                if use_bias:
                    nc.tensor.matmul(
                        ps_o[:, nsl],
                        ones[:],
                        b_sb[0:1, 3 * E + nn * 512: 3 * E + (nn + 1) * 512],
                        start=False, stop=True)
            o_sb = o2pool.tile([128, E], f32, tag="out")
            nc.scalar.copy(o_sb[:], ps_o[:])
            nc.sync.dma_start(out_d[tsl, :], o_sb[:])

    return _patch_bass(nc)


LAST_RESULTS = None


def kernel(**inputs) -> np.ndarray:
    global LAST_RESULTS
    from concourse import bass_utils

    qkv = np.asarray(inputs["QKV"], dtype=np.float32)
    is_causal = bool(int(np.asarray(inputs["is_causal"])))
    X = np.ascontiguousarray(qkv.reshape(TOK, E).astype(np.float16))
    wts = {n: np.ascontiguousarray(np.asarray(inputs[wn], dtype=np.float32).T.astype(np.float16))
           for n, wn in (("wq", "Wq"), ("wk", "Wk"), ("wv", "Wv"), ("wo", "Wo"))}
    # V projection emits d-major head layout: col d*16+g holds head g, dim d
    wts["wv"] = np.ascontiguousarray(
        wts["wv"].reshape(E, H, D).transpose(0, 2, 1).reshape(E, E))
    bias4 = np.ascontiguousarray(np.concatenate([
        np.asarray(inputs[b], dtype=np.float32) for b in ("bq", "bk", "bv", "bo")])[None, :].astype(np.float16))
    ident = np.eye(128, dtype=np.float16)

    use_bias = any(
        float(np.abs(np.asarray(inputs[b])).max()) != 0.0
        for b in ("bq", "bk", "bv", "bo"))
    key = (is_causal, use_bias)
    if key not in _cache:
        _cache[key] = _build(is_causal, use_bias)
    nc = _cache[key]

    in_maps = []
    for c in range(NCORES):
        xt_c = np.ascontiguousarray(X[c * TPC:(c + 1) * TPC].T)
        in_maps.append({"xt": xt_c, "bias4": bias4, "ident": ident, **wts})

    trace = bool(int(os.environ.get("BASSMHA_TRACE", "0")))
    res = bass_utils.run_bass_kernel_spmd(
        nc, in_maps, core_ids=list(range(NCORES)), trace=trace)
    LAST_RESULTS = res
    out = np.concatenate([res.results[c]["out"] for c in range(NCORES)], axis=0)
    return out.reshape(B, S, E)


if __name__ == "__main__":
    np.random.seed(0)
    fake = {
        "QKV": np.random.randn(B, S, E).astype(np.float32),
        "Wq": np.random.randn(E, E).astype(np.float32) * 0.02,
        "bq": np.zeros(E, np.float32),
        "Wk": np.random.randn(E, E).astype(np.float32) * 0.02,
        "bk": np.zeros(E, np.float32),
        "Wv": np.random.randn(E, E).astype(np.float32) * 0.02,
        "bv": np.zeros(E, np.float32),
        "Wo": np.random.randn(E, E).astype(np.float32) * 0.02,
        "bo": np.zeros(E, np.float32),
        "is_causal": 1,
    }
    o = kernel(**fake)
    print("kernel ok", o.shape, o.dtype, float(np.abs(o).mean()))


# revision 19
# speedup vs baseline: 1.1857x; 1.1857x over previous
"""Trainium2 Bass kernel for nn_MultiHeadAttention_90993177133622.

Math (from reference):
  Q = QKV @ Wq.T + bq   (same for K, V)            [B,S,E] -> view [B,S,H,D]
  P[b,s,h,g] = sum_d Q[b,s,h,d] K[b,s,g,d] * sqrt(D)   (per-token [H,H] attn)
  causal tril mask over [H,H], softmax over g
  out = (P @ V).reshape(B,S,E) @ Wo.T + bo

Sharding: data-parallel over the 16384 tokens across 8 cores (2048 each).
Device layout: token-major [128 tokens, E] tiles; projections run on the PE
(activation-stationary, float32r), per-token attention on DVE/ACT.
"""
import sys, os, types, ctypes, contextlib, json, math

sys.path.insert(0, "/opt/trn_rl_repo")
import numpy as np

B, S, E, H, D = 4, 4096, 1024, 16, 64
NCORES = 8
TOK = B * S                 # 16384
TPC = TOK // NCORES         # tokens per core: 2048
CHUNK = 128                 # tokens per tile
NCH = TPC // CHUNK          # 16 chunks per core
KC = E // 128               # 8 contraction chunks
SCALE = math.sqrt(D)        # reference MULTIPLIES by sqrt(D)
NEG = -1.0e30


# ---------------------------------------------------------------- infra shims
def _install_ntff_hook():
    """antenv.axon_hooks is missing in this image; provide it so
    run_bass_kernel_spmd(trace=True) can profile via libaxon_pjrt."""
    if "antenv.axon_hooks" in sys.modules:
        return
    mod = types.ModuleType("antenv.axon_hooks")
    state = {"hook": None}
    mod.set_axon_ntff_profile_hook = lambda h: state.__setitem__("hook", h)
    mod.get_axon_ntff_profile_hook = lambda: state["hook"]
    sys.modules["antenv.axon_hooks"] = mod
    try:
        lib = ctypes.CDLL("/opt/axon/libaxon_pjrt.so")
    except OSError:
        return
    if not hasattr(lib, "axon_start_nrt_profile"):
        return
    lib.axon_start_nrt_profile.argtypes = [ctypes.POINTER(ctypes.c_int64), ctypes.c_size_t]
    lib.axon_start_nrt_profile.restype = ctypes.c_int64
    lib.axon_stop_nrt_profile.argtypes = [ctypes.c_char_p]
    lib.axon_stop_nrt_profile.restype = ctypes.c_int64

    @contextlib.contextmanager
    def _hook(output_dir, device_ids):
        import jax
        jax.devices()
        if device_ids:
            ids = (ctypes.c_int64 * len(device_ids))(*device_ids)
            rc = lib.axon_start_nrt_profile(ids, len(device_ids))
        else:
            rc = lib.axon_start_nrt_profile(None, 0)
        if rc != 0:
            raise RuntimeError(f"axon_start_nrt_profile rc={rc}")
        try:
            yield
        finally:
            n = lib.axon_stop_nrt_profile(str(output_dir).encode())
            print(f"profile: {n} file(s) -> {output_dir}", file=sys.stderr)

    mod.set_axon_ntff_profile_hook(_hook)


_install_ntff_hook()

_MAX_WAITS = 2


def _split_waits_json(raw: bytes) -> bytes:
    """This walrus build rejects CTRL instructions with >2 sync waits; split
    extra waits off Drain/EventSemaphore into preceding wait-only ctrls."""
    j = json.loads(raw)
    for fn in j["functions"]:
        for bb in fn["blocks"]:
            out = []
            for inst in bb["instructions"]:
                si = inst.get("sync_info")
                waits = (si or {}).get("on_wait") or []
                limit = _MAX_WAITS if inst.get("opcode") == "EventSemaphore" else 1
                if len(waits) > limit:
                    head, tail = waits[:-limit], waits[-limit:]
                    for k in range(0, len(head), _MAX_WAITS):
                        out.append({
                            "debug": inst.get("debug", 0),
                            "engine": inst["engine"],
                            "ins": [], "outs": [],
                            "name": inst["name"] + f"_w{k}",
                            "opcode": "EventSemaphore",
                            "sync_info": {"on_update": [], "on_wait": head[k:k + _MAX_WAITS]},
                        })
                    si["on_wait"] = tail
                out.append(inst)
            bb["instructions"] = out
    return json.dumps(j).encode()


def _patch_bass(nc):
    orig = nc.to_json_bytes
    nc.to_json_bytes = lambda: _split_waits_json(orig())
    return nc


# ---------------------------------------------------------------- the program
_cache = {}


def _build(is_causal: bool, use_bias: bool = True):
    import concourse.bass as bass
    import concourse.tile as tile
    import concourse.mybir as mybir
    from contextlib import ExitStack

    f32 = mybir.dt.float32
    f16 = mybir.dt.float16
    Alu = mybir.AluOpType
    Act = mybir.ActivationFunctionType

    nc = bass.Bass("TRN2", target_bir_lowering=False, debug=False, enable_asserts=False)

    xt = nc.dram_tensor("xt", [E, TPC], f16, kind="ExternalInput").ap()
    ws = {n: nc.dram_tensor(n, [E, E], f16, kind="ExternalInput").ap()
          for n in ("wq", "wk", "wv", "wo")}
    bias4 = nc.dram_tensor("bias4", [1, 4 * E], f16, kind="ExternalInput").ap()
    ident = nc.dram_tensor("ident", [128, 128], f16, kind="ExternalInput").ap()
    out_d = nc.dram_tensor("out", [TPC, E], f32, kind="ExternalOutput").ap()

    with tile.TileContext(nc) as tc, ExitStack() as ctx:
        wpool = ctx.enter_context(tc.tile_pool(name="w", bufs=1))
        xpool = ctx.enter_context(tc.tile_pool(name="x", bufs=2))
        qkv = ctx.enter_context(tc.tile_pool(name="qkv", bufs=2))
        ppool = ctx.enter_context(tc.tile_pool(name="p", bufs=1))
        prodp = ctx.enter_context(tc.tile_pool(name="prod", bufs=2))
        treep = ctx.enter_context(tc.tile_pool(name="tree", bufs=2))
        treei = ctx.enter_context(tc.tile_pool(name="treei", bufs=1))
        _ttag = lambda n: f"ts{n}"
        stats = ctx.enter_context(tc.tile_pool(name="st", bufs=2))
        opool = ctx.enter_context(tc.tile_pool(name="o", bufs=1))
        o2pool = ctx.enter_context(tc.tile_pool(name="o2", bufs=2))
        psum = ctx.enter_context(tc.tile_pool(name="ps", bufs=2, space="PSUM"))
        psum1 = ctx.enter_context(tc.tile_pool(name="ps1", bufs=1, space="PSUM"))

        # resident constants
        w_sb = {}
        for n in ("wq", "wk", "wv", "wo"):
            t = wpool.tile([128, KC, E], f16, tag=f"w_{n}")
            nc.sync.dma_start(t[:], ws[n].rearrange("(k p) n -> p k n", p=128))
            w_sb[n] = t
        if use_bias:
            b_sb = wpool.tile([1, 4 * E], f16, tag="bias")
            nc.sync.dma_start(b_sb[:], bias4[:])
            ones = wpool.tile([1, CHUNK], f16, tag="ones")
            nc.vector.memset(ones[:], 1.0)
        id_sb = wpool.tile([128, 128], f16, tag="ident")
        nc.sync.dma_start(id_sb[:], ident[:])

        xt_r = xt.rearrange("(k p) t -> p k t", p=128)

        for ci in range(NCH):
            tsl = slice(ci * CHUNK, (ci + 1) * CHUNK)
            # ---- load X chunk (feature-major lhsT for all projections)
            x_sb = xpool.tile([128, KC, CHUNK], f16, tag="x")
            nc.sync.dma_start(x_sb[:], xt_r[:, :, tsl])

            # ---- Q,K,V projections -> token-major [128 tok, E] fp16
            sb = {}
            for pi, n in enumerate(("wq", "wk", "wv")):
                ps = psum.tile([128, E], f32, tag="proj")
                for k in range(KC):
                    for nn in range(2):
                        nsl = slice(nn * 512, (nn + 1) * 512)
                        nc.tensor.matmul(
                            ps[:, nsl],
                            x_sb[:, k, :],
                            w_sb[n][:, k, nsl],
                            start=(k == 0),
                            stop=(k == KC - 1 and not use_bias))
                if use_bias:
                    for nn in range(2):
                        nsl = slice(nn * 512, (nn + 1) * 512)
                        nc.tensor.matmul(
                            ps[:, nsl],
                            ones[:],
                            b_sb[0:1, pi * E + nn * 512: pi * E + (nn + 1) * 512],
                            start=False, stop=True)
                t = qkv.tile([128, E], f16, tag=n)
                nc.scalar.copy(t[:], ps[:])
                sb[n] = t
            q_sb, k_sb, v_sb = sb["wq"], sb["wk"], sb["wv"]

            # ---- attention (token-major, per-head causal-packed, fp16 TT ops)
            gmax = (lambda h: h + 1) if is_causal else (lambda h: H)
            NP = sum(gmax(h) for h in range(H))      # packed pair count
            off = [0] * (H + 1)
            for h in range(H):
                off[h + 1] = off[h] + gmax(h)

            def tt(eng, out, in0, in1, op):
                return eng.add_instruction(mybir.InstTensorTensor(
                    name=nc.get_next_instruction_name(), op=op,
                    ins=[eng.lower_ap(in0), eng.lower_ap(in1)],
                    outs=[eng.lower_ap(out)]))

            p_t = ppool.tile([128, H * H], f32, tag="praw")
            nc.gpsimd.memset(p_t[:], NEG)
            k_v = k_sb[:].rearrange("p (g d) -> p g d", g=H)
            qp = treep.tile([128, NP * D], f16, tag="big")
            for h in range(H):
                g = gmax(h)
                tt(nc.vector,
                   qp[:, off[h] * D:off[h + 1] * D].rearrange("p (g d) -> p g d", g=g),
                   q_sb[:, h * D:(h + 1) * D].unsqueeze(1).broadcast_to([128, g, D]),
                   k_v[:, :g, :], Alu.mult)
            # bulk pairwise tree over d: 64 -> 2
            lv = qp[:].rearrange("p (n d) -> p n d", n=NP)
            w = D
            for li in range(5):
                w //= 2
                nt = treei.tile([128, NP * w], f16, tag=_ttag(NP * w))
                tt(nc.vector,
                   nt[:].rearrange("p (n d) -> p n d", n=NP),
                   lv[:, :, 0:w], lv[:, :, w:2 * w], Alu.add)
                lv = nt[:].rearrange("p (n d) -> p n d", n=NP)
            for h in range(H):
                g = gmax(h)
                tt(nc.vector,
                   p_t[:, h * H:h * H + g].unsqueeze(2),
                   lv[:, off[h]:off[h + 1], 0:1], lv[:, off[h]:off[h + 1], 1:2],
                   Alu.add)

            p3 = p_t[:].rearrange("p (h g) -> p h g", h=H)
            mx = stats.tile([128, H], f32, tag="mx")
            nc.vector.tensor_reduce(mx[:], p3, mybir.AxisListType.X, Alu.max)
            p2_t = ppool.tile([128, H * H], f32, tag="psub")
            tt(nc.vector,
               p2_t[:].rearrange("p (h g) -> p h g", h=H),
               p3, mx[:].unsqueeze(2).broadcast_to([128, H, H]), Alu.subtract)
            ex_t = ppool.tile([128, H * H], f16, tag="pexp")
            nc.scalar.activation(ex_t[:], p2_t[:], Act.Exp, scale=float(SCALE))
            sm = stats.tile([128, H], f32, tag="sm")
            nc.vector.tensor_reduce(
                sm[:], ex_t[:].rearrange("p (h g) -> p h g", h=H),
                mybir.AxisListType.X, Alu.add)
            rc = stats.tile([128, H], f32, tag="rc")
            nc.vector.reciprocal(rc[:], sm[:])
            pn_t = ppool.tile([128, H * H], f16, tag="pnorm")
            tt(nc.vector,
               pn_t[:].rearrange("p (h g) -> p h g", h=H),
               ex_t[:].rearrange("p (h g) -> p h g", h=H),
               rc[:].unsqueeze(2).broadcast_to([128, H, H]), Alu.mult)

            attn = opool.tile([128, E], f16, tag="attn")
            v_dg = v_sb[:].rearrange("p (d g) -> p d g", g=H)  # V is d-major
            # padded head groups: head h in group with g-extent Gp (pn is 0 on
            # masked g, so padded products vanish); bulk pairwise tree per group
            if is_causal:
                groups = [(0, 8, 8), (8, 16, 16)]
            else:
                groups = [(0, 16, 16)]
            with nc.allow_low_precision("fp16 attn accumulation over 16 heads"):
                for h0, h1, Gp in groups:
                    nh = h1 - h0
                    gp = treep.tile([128, nh * D * Gp], f16, tag=("big" if Gp == 16 and is_causal else _ttag(nh * D * Gp)))
                    tt(nc.vector,
                       gp[:].rearrange("p (h d g) -> p h d g", h=nh, d=D),
                       pn_t[:, h0 * H:h1 * H]
                       .rearrange("p (h g) -> p h g", h=nh)[:, :, :Gp]
                       .unsqueeze(2).broadcast_to([128, nh, D, Gp]),
                       v_dg[:, :, :Gp].unsqueeze(1)
                       .broadcast_to([128, nh, D, Gp]),
                       Alu.mult)
                    lvv = gp[:].rearrange("p (n g) -> p n g", n=nh * D)
                    w2 = Gp
                    li = 0
                    while w2 > 2:
                        w2 //= 2
                        nt2 = treei.tile([128, nh * D * w2], f16, tag="i" + _ttag(nh * D * w2))
                        tt(nc.vector,
                           nt2[:].rearrange("p (n g) -> p n g", n=nh * D),
                           lvv[:, :, 0:w2], lvv[:, :, w2:2 * w2], Alu.add)
                        lvv = nt2[:].rearrange("p (n g) -> p n g", n=nh * D)
                        li += 1
                    tt(nc.vector,
                       attn[:, h0 * D:h1 * D].unsqueeze(2),
                       lvv[:, :, 0:1], lvv[:, :, 1:2], Alu.add)

            # ---- transpose attn to feature-major for the O projection
            ps_t = psum1.tile([128, E], f16, tag="tr")
            for j in range(KC):
                nc.tensor.transpose(
                    ps_t[:, j * 128:(j + 1) * 128],
                    attn[:, j * 128:(j + 1) * 128], id_sb[:])
            ao = opool.tile([128, KC, CHUNK], f16, tag="ao")
            nc.scalar.copy(ao[:], ps_t[:])

            # ---- O projection -> token-major out chunk
            ps_o = psum1.tile([128, E], f32, tag="oproj")
            for nn in range(2):
                nsl = slice(nn * 512, (nn + 1) * 512)
                for k in range(KC):
                    nc.tensor.matmul(
                        ps_o[:, nsl],
                        ao[:, k, :],
                        w_sb["wo"][:, k, nsl],
                        start=(k == 0),
                        stop=(k == KC - 1 and not use_bias))
                if use_bias:
                    nc.tensor.matmul(
                        ps_o[:, nsl],
                        ones[:],
                        b_sb[0:1, 3 * E + nn * 512: 3 * E + (nn + 1) * 512],
                        start=False, stop=True)
            o_sb = o2pool.tile([128, E], f32, tag="out")
            nc.scalar.copy(o_sb[:], ps_o[:])
            nc.sync.dma_start(out_d[tsl, :], o_sb[:])

    return _patch_bass(nc)


LAST_RESULTS = None


def kernel(**inputs) -> np.ndarray:
    global LAST_RESULTS
    from concourse import bass_utils

    qkv = np.asarray(inputs["QKV"], dtype=np.float32)
    is_causal = bool(int(np.asarray(inputs["is_causal"])))
    X = np.ascontiguousarray(qkv.reshape(TOK, E).astype(np.float16))
    wts = {n: np.ascontiguousarray(np.asarray(inputs[wn], dtype=np.float32).T.astype(np.float16))
           for n, wn in (("wq", "Wq"), ("wk", "Wk"), ("wv", "Wv"), ("wo", "Wo"))}
    # V projection emits d-major head layout: col d*16+g holds head g, dim d
    wts["wv"] = np.ascontiguousarray(
        wts["wv"].reshape(E, H, D).transpose(0, 2, 1).reshape(E, E))
    bias4 = np.ascontiguousarray(np.concatenate([
        np.asarray(inputs[b], dtype=np.float32) for b in ("bq", "bk", "bv", "bo")])[None, :].astype(np.float16))
    ident = np.eye(128, dtype=np.float16)

    use_bias = any(
        float(np.abs(np.asarray(inputs[b])).max()) != 0.0
        for b in ("bq", "bk", "bv", "bo"))
    key = (is_causal, use_bias)
    if key not in _cache:
        _cache[key] = _build(is_causal, use_bias)
    nc = _cache[key]

    in_maps = []
    for c in range(NCORES):
        xt_c = np.ascontiguousarray(X[c * TPC:(c + 1) * TPC].T)
        in_maps.append({"xt": xt_c, "bias4": bias4, "ident": ident, **wts})

    trace = bool(int(os.environ.get("BASSMHA_TRACE", "0")))
    res = bass_utils.run_bass_kernel_spmd(
        nc, in_maps, core_ids=list(range(NCORES)), trace=trace)
    LAST_RESULTS = res
    out = np.concatenate([res.results[c]["out"] for c in range(NCORES)], axis=0)
    return out.reshape(B, S, E)


if __name__ == "__main__":
    np.random.seed(0)
    fake = {
        "QKV": np.random.randn(B, S, E).astype(np.float32),
        "Wq": np.random.randn(E, E).astype(np.float32) * 0.02,
        "bq": np.zeros(E, np.float32),
        "Wk": np.random.randn(E, E).astype(np.float32) * 0.02,
        "bk": np.zeros(E, np.float32),
        "Wv": np.random.randn(E, E).astype(np.float32) * 0.02,
        "bv": np.zeros(E, np.float32),
        "Wo": np.random.randn(E, E).astype(np.float32) * 0.02,
        "bo": np.zeros(E, np.float32),
        "is_causal": 1,
    }
    o = kernel(**fake)
    print("kernel ok", o.shape, o.dtype, float(np.abs(o).mean()))


# revision 20
# speedup vs baseline: 1.1938x; 1.0068x over previous
"""Trainium2 Bass kernel for nn_MultiHeadAttention_90993177133622.

Math (from reference):
  Q = QKV @ Wq.T + bq   (same for K, V)            [B,S,E] -> view [B,S,H,D]
  P[b,s,h,g] = sum_d Q[b,s,h,d] K[b,s,g,d] * sqrt(D)   (per-token [H,H] attn)
  causal tril mask over [H,H], softmax over g
  out = (P @ V).reshape(B,S,E) @ Wo.T + bo

Sharding: data-parallel over the 16384 tokens across 8 cores (2048 each).
Device layout: token-major [128 tokens, E] tiles; projections run on the PE
(activation-stationary, float32r), per-token attention on DVE/ACT.
"""
import sys, os, types, ctypes, contextlib, json, math

sys.path.insert(0, "/opt/trn_rl_repo")
import numpy as np

B, S, E, H, D = 4, 4096, 1024, 16, 64
NCORES = 8
TOK = B * S                 # 16384
TPC = TOK // NCORES         # tokens per core: 2048
CHUNK = 128                 # tokens per tile
NCH = TPC // CHUNK          # 16 chunks per core
KC = E // 128               # 8 contraction chunks
SCALE = math.sqrt(D)        # reference MULTIPLIES by sqrt(D)
NEG = -1.0e30


# ---------------------------------------------------------------- infra shims
def _install_ntff_hook():
    """antenv.axon_hooks is missing in this image; provide it so
    run_bass_kernel_spmd(trace=True) can profile via libaxon_pjrt."""
    if "antenv.axon_hooks" in sys.modules:
        return
    mod = types.ModuleType("antenv.axon_hooks")
    state = {"hook": None}
    mod.set_axon_ntff_profile_hook = lambda h: state.__setitem__("hook", h)
    mod.get_axon_ntff_profile_hook = lambda: state["hook"]
    sys.modules["antenv.axon_hooks"] = mod
    try:
        lib = ctypes.CDLL("/opt/axon/libaxon_pjrt.so")
    except OSError:
        return
    if not hasattr(lib, "axon_start_nrt_profile"):
        return
    lib.axon_start_nrt_profile.argtypes = [ctypes.POINTER(ctypes.c_int64), ctypes.c_size_t]
    lib.axon_start_nrt_profile.restype = ctypes.c_int64
    lib.axon_stop_nrt_profile.argtypes = [ctypes.c_char_p]
    lib.axon_stop_nrt_profile.restype = ctypes.c_int64

    @contextlib.contextmanager
    def _hook(output_dir, device_ids):
        import jax
        jax.devices()
        if device_ids:
            ids = (ctypes.c_int64 * len(device_ids))(*device_ids)
            rc = lib.axon_start_nrt_profile(ids, len(device_ids))
        else:
            rc = lib.axon_start_nrt_profile(None, 0)
        if rc != 0:
            raise RuntimeError(f"axon_start_nrt_profile rc={rc}")
        try:
            yield
        finally:
            n = lib.axon_stop_nrt_profile(str(output_dir).encode())
            print(f"profile: {n} file(s) -> {output_dir}", file=sys.stderr)

    mod.set_axon_ntff_profile_hook(_hook)


_install_ntff_hook()

_MAX_WAITS = 2


def _split_waits_json(raw: bytes) -> bytes:
    """This walrus build rejects CTRL instructions with >2 sync waits; split
    extra waits off Drain/EventSemaphore into preceding wait-only ctrls."""
    j = json.loads(raw)
    for fn in j["functions"]:
        for bb in fn["blocks"]:
            out = []
            for inst in bb["instructions"]:
                si = inst.get("sync_info")
                waits = (si or {}).get("on_wait") or []
                limit = _MAX_WAITS if inst.get("opcode") == "EventSemaphore" else 1
                if len(waits) > limit:
                    head, tail = waits[:-limit], waits[-limit:]
                    for k in range(0, len(head), _MAX_WAITS):
                        out.append({
                            "debug": inst.get("debug", 0),
                            "engine": inst["engine"],
                            "ins": [], "outs": [],
                            "name": inst["name"] + f"_w{k}",
                            "opcode": "EventSemaphore",
                            "sync_info": {"on_update": [], "on_wait": head[k:k + _MAX_WAITS]},
                        })
                    si["on_wait"] = tail
                out.append(inst)
            bb["instructions"] = out
    return json.dumps(j).encode()


def _patch_bass(nc):
    orig = nc.to_json_bytes
    nc.to_json_bytes = lambda: _split_waits_json(orig())
    return nc


# ---------------------------------------------------------------- the program
_cache = {}


def _build(is_causal: bool, use_bias: bool = True):
    import concourse.bass as bass
    import concourse.tile as tile
    import concourse.mybir as mybir
    from contextlib import ExitStack

    f32 = mybir.dt.float32
    f16 = mybir.dt.float16
    Alu = mybir.AluOpType
    Act = mybir.ActivationFunctionType

    nc = bass.Bass("TRN2", target_bir_lowering=False, debug=False, enable_asserts=False)

    xt = nc.dram_tensor("xt", [E, TPC], f16, kind="ExternalInput").ap()
    ws = {n: nc.dram_tensor(n, [E, E], f16, kind="ExternalInput").ap()
          for n in ("wq", "wk", "wv", "wo")}
    bias4 = nc.dram_tensor("bias4", [1, 4 * E], f16, kind="ExternalInput").ap()
    ident = nc.dram_tensor("ident", [128, 128], f16, kind="ExternalInput").ap()
    out_d = nc.dram_tensor("out", [TPC, E], f32, kind="ExternalOutput").ap()

    with tile.TileContext(nc) as tc, ExitStack() as ctx:
        wpool = ctx.enter_context(tc.tile_pool(name="w", bufs=1))
        xpool = ctx.enter_context(tc.tile_pool(name="x", bufs=2))
        qkv = ctx.enter_context(tc.tile_pool(name="qkv", bufs=3))
        ppool = ctx.enter_context(tc.tile_pool(name="p", bufs=2))
        prodp = ctx.enter_context(tc.tile_pool(name="prod", bufs=2))
        treep = ctx.enter_context(tc.tile_pool(name="tree", bufs=2))
        treei = ctx.enter_context(tc.tile_pool(name="treei", bufs=1))
        _ttag = lambda n: f"ts{n}"
        stats = ctx.enter_context(tc.tile_pool(name="st", bufs=2))
        opool = ctx.enter_context(tc.tile_pool(name="o", bufs=1))
        o2pool = ctx.enter_context(tc.tile_pool(name="o2", bufs=2))
        psum = ctx.enter_context(tc.tile_pool(name="ps", bufs=2, space="PSUM"))
        psum1 = ctx.enter_context(tc.tile_pool(name="ps1", bufs=1, space="PSUM"))

        # resident constants
        w_sb = {}
        for n in ("wq", "wk", "wv", "wo"):
            t = wpool.tile([128, KC, E], f16, tag=f"w_{n}")
            nc.sync.dma_start(t[:], ws[n].rearrange("(k p) n -> p k n", p=128))
            w_sb[n] = t
        if use_bias:
            b_sb = wpool.tile([1, 4 * E], f16, tag="bias")
            nc.sync.dma_start(b_sb[:], bias4[:])
            ones = wpool.tile([1, CHUNK], f16, tag="ones")
            nc.vector.memset(ones[:], 1.0)
        id_sb = wpool.tile([128, 128], f16, tag="ident")
        nc.sync.dma_start(id_sb[:], ident[:])

        xt_r = xt.rearrange("(k p) t -> p k t", p=128)

        for ci in range(NCH):
            tsl = slice(ci * CHUNK, (ci + 1) * CHUNK)
            # ---- load X chunk (feature-major lhsT for all projections)
            x_sb = xpool.tile([128, KC, CHUNK], f16, tag="x")
            nc.sync.dma_start(x_sb[:], xt_r[:, :, tsl])

            # ---- Q,K,V projections -> token-major [128 tok, E] fp16
            sb = {}
            for pi, n in enumerate(("wq", "wk", "wv")):
                ps = psum.tile([128, E], f32, tag="proj")
                for k in range(KC):
                    for nn in range(2):
                        nsl = slice(nn * 512, (nn + 1) * 512)
                        nc.tensor.matmul(
                            ps[:, nsl],
                            x_sb[:, k, :],
                            w_sb[n][:, k, nsl],
                            start=(k == 0),
                            stop=(k == KC - 1 and not use_bias))
                if use_bias:
                    for nn in range(2):
                        nsl = slice(nn * 512, (nn + 1) * 512)
                        nc.tensor.matmul(
                            ps[:, nsl],
                            ones[:],
                            b_sb[0:1, pi * E + nn * 512: pi * E + (nn + 1) * 512],
                            start=False, stop=True)
                t = qkv.tile([128, E], f16, tag=n)
                nc.scalar.copy(t[:], ps[:])
                sb[n] = t
            q_sb, k_sb, v_sb = sb["wq"], sb["wk"], sb["wv"]

            # ---- attention (token-major, per-head causal-packed, fp16 TT ops)
            gmax = (lambda h: h + 1) if is_causal else (lambda h: H)
            NP = sum(gmax(h) for h in range(H))      # packed pair count
            off = [0] * (H + 1)
            for h in range(H):
                off[h + 1] = off[h] + gmax(h)

            def tt(eng, out, in0, in1, op):
                return eng.add_instruction(mybir.InstTensorTensor(
                    name=nc.get_next_instruction_name(), op=op,
                    ins=[eng.lower_ap(in0), eng.lower_ap(in1)],
                    outs=[eng.lower_ap(out)]))

            p_t = ppool.tile([128, H * H], f32, tag="praw")
            nc.gpsimd.memset(p_t[:], NEG)
            k_v = k_sb[:].rearrange("p (g d) -> p g d", g=H)
            qp = treep.tile([128, NP * D], f16, tag="big")
            for h in range(H):
                g = gmax(h)
                tt(nc.vector,
                   qp[:, off[h] * D:off[h + 1] * D].rearrange("p (g d) -> p g d", g=g),
                   q_sb[:, h * D:(h + 1) * D].unsqueeze(1).broadcast_to([128, g, D]),
                   k_v[:, :g, :], Alu.mult)
            # bulk pairwise tree over d: 64 -> 2
            lv = qp[:].rearrange("p (n d) -> p n d", n=NP)
            w = D
            for li in range(5):
                w //= 2
                nt = treei.tile([128, NP * w], f16, tag=_ttag(NP * w))
                tt(nc.vector,
                   nt[:].rearrange("p (n d) -> p n d", n=NP),
                   lv[:, :, 0:w], lv[:, :, w:2 * w], Alu.add)
                lv = nt[:].rearrange("p (n d) -> p n d", n=NP)
            for h in range(H):
                g = gmax(h)
                tt(nc.vector,
                   p_t[:, h * H:h * H + g].unsqueeze(2),
                   lv[:, off[h]:off[h + 1], 0:1], lv[:, off[h]:off[h + 1], 1:2],
                   Alu.add)

            p3 = p_t[:].rearrange("p (h g) -> p h g", h=H)
            mx = stats.tile([128, H], f32, tag="mx")
            nc.vector.tensor_reduce(mx[:], p3, mybir.AxisListType.X, Alu.max)
            p2_t = ppool.tile([128, H * H], f32, tag="psub")
            tt(nc.vector,
               p2_t[:].rearrange("p (h g) -> p h g", h=H),
               p3, mx[:].unsqueeze(2).broadcast_to([128, H, H]), Alu.subtract)
            ex_t = ppool.tile([128, H * H], f16, tag="pexp")
            nc.scalar.activation(ex_t[:], p2_t[:], Act.Exp, scale=float(SCALE))
            sm = stats.tile([128, H], f32, tag="sm")
            nc.vector.tensor_reduce(
                sm[:], ex_t[:].rearrange("p (h g) -> p h g", h=H),
                mybir.AxisListType.X, Alu.add)
            rc = stats.tile([128, H], f32, tag="rc")
            nc.vector.reciprocal(rc[:], sm[:])
            pn_t = ppool.tile([128, H * H], f16, tag="pnorm")
            tt(nc.vector,
               pn_t[:].rearrange("p (h g) -> p h g", h=H),
               ex_t[:].rearrange("p (h g) -> p h g", h=H),
               rc[:].unsqueeze(2).broadcast_to([128, H, H]), Alu.mult)

            attn = opool.tile([128, E], f16, tag="attn")
            v_dg = v_sb[:].rearrange("p (d g) -> p d g", g=H)  # V is d-major
            # padded head groups: head h in group with g-extent Gp (pn is 0 on
            # masked g, so padded products vanish); bulk pairwise tree per group
            if is_causal:
                groups = [(0, 8, 8), (8, 16, 16)]
            else:
                groups = [(0, 16, 16)]
            with nc.allow_low_precision("fp16 attn accumulation over 16 heads"):
                for h0, h1, Gp in groups:
                    nh = h1 - h0
                    gp = treep.tile([128, nh * D * Gp], f16, tag=("big" if Gp == 16 and is_causal else _ttag(nh * D * Gp)))
                    tt(nc.vector,
                       gp[:].rearrange("p (h d g) -> p h d g", h=nh, d=D),
                       pn_t[:, h0 * H:h1 * H]
                       .rearrange("p (h g) -> p h g", h=nh)[:, :, :Gp]
                       .unsqueeze(2).broadcast_to([128, nh, D, Gp]),
                       v_dg[:, :, :Gp].unsqueeze(1)
                       .broadcast_to([128, nh, D, Gp]),
                       Alu.mult)
                    lvv = gp[:].rearrange("p (n g) -> p n g", n=nh * D)
                    w2 = Gp
                    li = 0
                    while w2 > 2:
                        w2 //= 2
                        nt2 = treei.tile([128, nh * D * w2], f16, tag="i" + _ttag(nh * D * w2))
                        tt(nc.vector,
                           nt2[:].rearrange("p (n g) -> p n g", n=nh * D),
                           lvv[:, :, 0:w2], lvv[:, :, w2:2 * w2], Alu.add)
                        lvv = nt2[:].rearrange("p (n g) -> p n g", n=nh * D)
                        li += 1
                    tt(nc.vector,
                       attn[:, h0 * D:h1 * D].unsqueeze(2),
                       lvv[:, :, 0:1], lvv[:, :, 1:2], Alu.add)

            # ---- transpose attn to feature-major for the O projection
            ps_t = psum1.tile([128, E], f16, tag="tr")
            for j in range(KC):
                nc.tensor.transpose(
                    ps_t[:, j * 128:(j + 1) * 128],
                    attn[:, j * 128:(j + 1) * 128], id_sb[:])
            ao = opool.tile([128, KC, CHUNK], f16, tag="ao")
            nc.scalar.copy(ao[:], ps_t[:])

            # ---- O projection -> token-major out chunk
            ps_o = psum1.tile([128, E], f32, tag="oproj")
            for nn in range(2):
                nsl = slice(nn * 512, (nn + 1) * 512)
                for k in range(KC):
                    nc.tensor.matmul(
                        ps_o[:, nsl],
                        ao[:, k, :],
                        w_sb["wo"][:, k, nsl],
                        start=(k == 0),
                        stop=(k == KC - 1 and not use_bias))
                if use_bias:
                    nc.tensor.matmul(
                        ps_o[:, nsl],
                        ones[:],
                        b_sb[0:1, 3 * E + nn * 512: 3 * E + (nn + 1) * 512],
                        start=False, stop=True)
            o_sb = o2pool.tile([128, E], f32, tag="out")
            nc.scalar.copy(o_sb[:], ps_o[:])
            nc.sync.dma_start(out_d[tsl, :], o_sb[:])

    return _patch_bass(nc)


LAST_RESULTS = None


def kernel(**inputs) -> np.ndarray:
    global LAST_RESULTS
    from concourse import bass_utils

    qkv = np.asarray(inputs["QKV"], dtype=np.float32)
    is_causal = bool(int(np.asarray(inputs["is_causal"])))
    X = np.ascontiguousarray(qkv.reshape(TOK, E).astype(np.float16))
    wts = {n: np.ascontiguousarray(np.asarray(inputs[wn], dtype=np.float32).T.astype(np.float16))
           for n, wn in (("wq", "Wq"), ("wk", "Wk"), ("wv", "Wv"), ("wo", "Wo"))}
    # V projection emits d-major head layout: col d*16+g holds head g, dim d
    wts["wv"] = np.ascontiguousarray(
        wts["wv"].reshape(E, H, D).transpose(0, 2, 1).reshape(E, E))
    bias4 = np.ascontiguousarray(np.concatenate([
        np.asarray(inputs[b], dtype=np.float32) for b in ("bq", "bk", "bv", "bo")])[None, :].astype(np.float16))
    ident = np.eye(128, dtype=np.float16)

    use_bias = any(
        float(np.abs(np.asarray(inputs[b])).max()) != 0.0
        for b in ("bq", "bk", "bv", "bo"))
    key = (is_causal, use_bias)
    if key not in _cache:
        _cache[key] = _build(is_causal, use_bias)
    nc = _cache[key]

    in_maps = []
    for c in range(NCORES):
        xt_c = np.ascontiguousarray(X[c * TPC:(c + 1) * TPC].T)
        in_maps.append({"xt": xt_c, "bias4": bias4, "ident": ident, **wts})

    trace = bool(int(os.environ.get("BASSMHA_TRACE", "0")))
    res = bass_utils.run_bass_kernel_spmd(
        nc, in_maps, core_ids=list(range(NCORES)), trace=trace)
    LAST_RESULTS = res
    out = np.concatenate([res.results[c]["out"] for c in range(NCORES)], axis=0)
    return out.reshape(B, S, E)


if __name__ == "__main__":
    np.random.seed(0)
    fake = {
        "QKV": np.random.randn(B, S, E).astype(np.float32),
        "Wq": np.random.randn(E, E).astype(np.float32) * 0.02,
        "bq": np.zeros(E, np.float32),
        "Wk": np.random.randn(E, E).astype(np.float32) * 0.02,
        "bk": np.zeros(E, np.float32),
        "Wv": np.random.randn(E, E).astype(np.float32) * 0.02,
        "bv": np.zeros(E, np.float32),
        "Wo": np.random.randn(E, E).astype(np.float32) * 0.02,
        "bo": np.zeros(E, np.float32),
        "is_causal": 1,
    }
    o = kernel(**fake)
    print("kernel ok", o.shape, o.dtype, float(np.abs(o).mean()))
